# revision 20
# baseline (speedup 1.0000x reference)
"""Segment-mean (graph pooling) kernel for Trainium2, 8 NeuronCores.

reference semantics:
    sums   = segment_sum(node_h, node_batch, num_segments=G)
    counts = segment_sum(ones(N), node_batch, G)
    out    = sums / max(counts, 1)[:, None]

node_batch is sorted, so segments are contiguous row runs. Sharding:
core c owns segments [128c, 128(c+1)) and streams the node rows that
cover them (a uniform T tiles per core; rows outside the core's
segment range contribute to nothing).

Because rows are sorted, a 128-row tile intersects at most 2 segments
(segments here have ~1953 rows each; host falls back if not). The
reduction runs in two phases so no full [rows, segs] one-hot is ever
built (in the v2 version its DVE is_equal build was the bottleneck at
~265us busy vs the ~180us DMA floor):

  Phase 1 (per tile): matmul with lhsT = the [128 rows, 128 d] bf16
  data tile (stationary) and rhs = [128 rows, 2] 0/1 masks (first-run
  rows / second-run rows, packed into the streamed slab right after
  the data tiles). Out = the tile's two run-sums as two f32 columns
  of a per-64-tile [128 d, 128 runs] PSUM block (tile j -> cols 2j).
  Matmul PSUM writes must start at partition base 0/32/64, so the
  block is built transposed ([d, runs]) and flipped back in phase 2.

  Phase 2 (per 64-tile block, staggered one step per block boundary
  so no engine waits on another): cast block PSUM->SBUF bf16, PE
  transpose via identity -> [runs, d] PSUM, cast -> SBUF bf16, build
  a [128 runs, 128 segs] one-hot from a [128,1] runseg column via one
  DVE is_equal, and accumulate oh.T @ runsumsT into the segment
  accumulator with one matmul. 31 blocks -> ~20us total DVE work.

node_h is cast to bf16 on the host (the ~2^-8 per-element quantization
noise averages down to ~2e-3 relative error on the segment means, well
inside the 2e-2 gate) so HBM traffic is half of f32. Epilogue scales
by 1/max(count,1) computed on host.

The per-instruction ISA limit of ONE semaphore wait shapes the
synchronization: slab DMAs run on the sync (HWDGE) queue, and tiny
gpsimd memset carriers chained on the gpsimd FIFO absorb the
cross-engine WAR waits for slab-buffer and PSUM-block reuse so every
hot-loop instruction needs at most one semaphore wait.
"""

import os

import numpy as np
import ml_dtypes

BF16 = ml_dtypes.bfloat16
P = 128  # partitions / rows per tile / segments per core
D = 128  # feature dim
G = 1024  # num segments
N_CORES = 8
SLAB = 64  # node-tiles per DMA slab
BLK = 64  # tiles per run-sum PSUM block (2 runs/tile -> 128 run cols)
SLABS_PER_BLK = BLK // SLAB
SLAB_COLS = SLAB * D + 2 * SLAB  # data tiles + [128,2] mask per tile
SENTINEL = 200.0  # run seg id outside [0, 128) -> dropped by one-hot
PSUM_BLK_BUFS = 4

_prog_cache: dict[int, object] = {}
LAST_RESULT = None  # BassKernelResults of the most recent device run


def _np_fallback(node_h, node_batch, num_graphs):
    node_h = np.asarray(node_h, dtype=np.float32)
    nb = np.asarray(node_batch).astype(np.int64)
    ng = int(num_graphs)
    sums = np.zeros((ng, node_h.shape[1]), dtype=np.float32)
    np.add.at(sums, nb, node_h)
    counts = np.bincount(nb, minlength=ng).astype(np.float32)
    return sums / np.maximum(counts, 1.0)[:, None]


def _build_program(T: int):
    import concourse.bacc as bacc
    import concourse.mybir as mybir
    import concourse.tile as tile
    from concourse.tile import add_dep_helper

    bf16 = mybir.dt.bfloat16
    f32 = mybir.dt.float32

    assert T % BLK == 0
    n_slabs = T // SLAB
    NB = T // BLK

    nc = bacc.Bacc(None)
    h_in = nc.dram_tensor("h", [P, n_slabs * SLAB_COLS], bf16, kind="ExternalInput")
    idx_in = nc.dram_tensor("idx", [P, 2 * P + NB], bf16, kind="ExternalInput")
    recip_in = nc.dram_tensor("recip", [P, 1], f32, kind="ExternalInput")
    out_t = nc.dram_tensor("out", [P, D], f32, kind="ExternalOutput")

    with tile.TileContext(nc) as tc:
        with (
            tc.tile_pool(name="const", bufs=1) as constp,
            tc.tile_pool(name="scr", bufs=max(1, n_slabs)) as scrp,
            tc.tile_pool(name="scr2", bufs=max(1, NB)) as scr2p,
            tc.tile_pool(name="slabs", bufs=7) as slabp,
            tc.tile_pool(name="rs", bufs=3) as rsp,
            tc.tile_pool(name="rst", bufs=3) as rstp,
            tc.tile_pool(name="oh2", bufs=4) as oh2p,
            tc.tile_pool(name="pblk", bufs=PSUM_BLK_BUFS, space="PSUM") as pblkp,
            tc.tile_pool(name="ptr", bufs=2, space="PSUM") as ptrp,
            tc.tile_pool(name="pacc", bufs=1, space="PSUM") as paccp,
            tc.tile_pool(name="outp", bufs=1) as outp,
        ):
            idx_sb = constp.tile([P, 2 * P + NB], bf16)
            nc.sync.dma_start(idx_sb[:], idx_in[:])
            recip_sb = constp.tile([P, 1], f32)
            nc.sync.dma_start(recip_sb[:], recip_in[:])
            iota_sb = idx_sb[:, 0:P]
            ident_sb = idx_sb[:, P : 2 * P]

            acc = paccp.tile([P, D], f32)

            last_mm = {}  # slab idx -> last phase-1 matmul (slab WAR)
            cast1_ins = {}  # block idx -> PSUM->SBUF copy (psum blk WAR)
            rs_tiles = {}
            rst_tiles = {}
            oh2_tiles = {}
            blk_tile = None

            def block_boundary(k):
                """Emitted right after the last phase-1 mm of block k."""
                b_tr = k - 1
                b_mm2 = k - 2
                rs_k = rsp.tile([P, 2 * BLK], bf16)
                cast1_ins[k] = nc.vector.tensor_copy(out=rs_k[:], in_=blk_tile[:])
                rs_tiles[k] = rs_k
                oh2 = oh2p.tile([P, P], bf16)
                nc.vector.tensor_tensor(
                    out=oh2[:],
                    in0=iota_sb,
                    in1=idx_sb[:, 2 * P + k : 2 * P + k + 1].to_broadcast([P, P]),
                    op=mybir.AluOpType.is_equal,
                )
                oh2_tiles[k] = oh2
                if b_tr >= 0:
                    rsT_ps = ptrp.tile([P, 2 * BLK], bf16)
                    nc.tensor.transpose(rsT_ps[:], rs_tiles.pop(b_tr)[:], ident_sb)
                    rsT_sb = rstp.tile([P, 2 * BLK], bf16)
                    nc.vector.tensor_copy(out=rsT_sb[:], in_=rsT_ps[:])
                    rst_tiles[b_tr] = rsT_sb
                if b_mm2 >= 0:
                    nc.tensor.matmul(
                        out=acc[:],
                        lhsT=oh2_tiles.pop(b_mm2)[:],
                        rhs=rst_tiles.pop(b_mm2)[:],
                        start=(b_mm2 == 0),
                        stop=(b_mm2 == NB - 1),
                    )

            for g in range(n_slabs):
                slab = slabp.tile([P, SLAB_COLS], bf16)
                # both carriers run on gpsimd (FIFO): waiting on the later
                # one implies the earlier completed, so the sync-queue DMA
                # needs a single cross-engine semaphore wait
                car = None
                if g >= 7:
                    scr = scrp.tile([1, 2], f32, name="scr")
                    car = nc.gpsimd.memset(scr[:], 0.0)
                    add_dep_helper(
                        car.ins, last_mm[g - 7].ins, True, "slab WAR carrier"
                    )
                b_new = g // SLABS_PER_BLK
                if g % SLABS_PER_BLK == 0 and b_new >= PSUM_BLK_BUFS:
                    scr2 = scr2p.tile([1, 2], f32, name="scr2")
                    car2 = nc.gpsimd.memset(scr2[:], 0.0)
                    add_dep_helper(
                        car2.ins,
                        cast1_ins[b_new - PSUM_BLK_BUFS].ins,
                        True,
                        "psum blk WAR carrier",
                    )
                    car = car2
                dma = nc.sync.dma_start(
                    slab[:], h_in[:, g * SLAB_COLS : (g + 1) * SLAB_COLS]
                )
                if car is not None:
                    add_dep_helper(dma.ins, car.ins, True, "dma after carriers")

                for i in range(SLAB):
                    t = g * SLAB + i
                    b = t // BLK
                    j = t % BLK
                    if j == 0:
                        blk_tile = pblkp.tile([P, 2 * BLK], f32)
                    mm = nc.tensor.matmul(
                        out=blk_tile[:, 2 * j : 2 * j + 2],
                        lhsT=slab[:, i * D : (i + 1) * D],
                        rhs=slab[:, SLAB * D + 2 * i : SLAB * D + 2 * i + 2],
                        start=True,
                        stop=True,
                    )
                    if i == SLAB - 1:
                        last_mm[g] = mm
                    if j == BLK - 1:
                        block_boundary(b)

            # flush the staggered pipeline: transpose/cast for NB-1, then
            # the last two accumulate matmuls
            b_tr = NB - 1
            rsT_ps = ptrp.tile([P, 2 * BLK], bf16)
            nc.tensor.transpose(rsT_ps[:], rs_tiles.pop(b_tr)[:], ident_sb)
            rsT_sb = rstp.tile([P, 2 * BLK], bf16)
            nc.vector.tensor_copy(out=rsT_sb[:], in_=rsT_ps[:])
            rst_tiles[b_tr] = rsT_sb
            for b_mm2 in (NB - 2, NB - 1):
                nc.tensor.matmul(
                    out=acc[:],
                    lhsT=oh2_tiles.pop(b_mm2)[:],
                    rhs=rst_tiles.pop(b_mm2)[:],
                    start=(b_mm2 == 0),
                    stop=(b_mm2 == NB - 1),
                )

            res = outp.tile([P, D], f32)
            nc.vector.tensor_tensor(
                out=res[:],
                in0=acc[:],
                in1=recip_sb[:, 0:1].to_broadcast([P, D]),
                op=mybir.AluOpType.mult,
            )
            nc.sync.dma_start(out_t[:], res[:])

    nc.finalize()
    return nc


def kernel(node_h, node_batch, num_graphs):
    global LAST_RESULT
    node_h = np.asarray(node_h)
    nb = np.asarray(node_batch)
    ng = int(num_graphs)

    N = node_h.shape[0]
    if (
        ng != G
        or node_h.ndim != 2
        or node_h.shape[1] != D
        or nb.shape != (N,)
        or N % P != 0
        or N // P < 4 * SLAB
        or np.any(nb[:-1] > nb[1:])
        or nb[0] < 0
        or nb[-1] >= G
    ):
        return _np_fallback(node_h, node_batch, num_graphs)

    node_h = np.ascontiguousarray(node_h, dtype=np.float32)
    nb = nb.astype(np.int64)

    n_tiles = N // P
    seg_per_core = G // N_CORES
    counts = np.bincount(nb, minlength=G)
    bounds = np.concatenate([[0], np.cumsum(counts)])
    starts = bounds[np.arange(N_CORES) * seg_per_core]
    ends = bounds[(np.arange(N_CORES) + 1) * seg_per_core]
    lo_t = starts // P
    hi_t = -(-ends // P)
    span = int((hi_t - lo_t).max())
    T = ((span + BLK - 1) // BLK) * BLK
    if T > n_tiles:
        return _np_fallback(node_h, node_batch, num_graphs)
    lo = np.minimum(lo_t, n_tiles - T).astype(np.int64)

    # any 128-row tile anywhere spanning >2 distinct segments breaks the
    # 2-runs-per-tile scheme -> fall back (cannot happen for ~1953-row
    # segments)
    seg_all = nb.reshape(n_tiles, P)
    d = seg_all[:, -1] - seg_all[:, 0]
    if np.any(d > 1):
        for t in np.where(d > 1)[0]:
            if len(np.unique(seg_all[t])) > 2:
                return _np_fallback(node_h, node_batch, num_graphs)

    n_slabs = T // SLAB
    NB = T // BLK
    rowidx = np.arange(P)

    in_maps = []
    for c in range(N_CORES):
        r0 = int(lo[c]) * P
        r1 = r0 + T * P
        data = node_h[r0:r1].astype(BF16).reshape(T, P, D)

        seg = (nb[r0:r1] - c * seg_per_core).reshape(T, P)
        first = seg[:, 0]
        last = seg[:, -1]
        k = (seg == first[:, None]).sum(axis=1)  # rows in first run
        mask0 = (rowidx[None, :] < k[:, None]).astype(BF16)  # [T, P]
        masks = np.empty((T, P, 2), dtype=BF16)
        masks[:, :, 0] = mask0
        masks[:, :, 1] = (1.0 - mask0.astype(np.float32)).astype(BF16)

        in_range_f = (first >= 0) & (first < P)
        in_range_l = (last >= 0) & (last < P)
        runseg = np.empty(2 * T, dtype=np.float32)
        runseg[0::2] = np.where(in_range_f, first, SENTINEL)
        runseg[1::2] = np.where((last != first) & in_range_l, last, SENTINEL)
        runsegT = runseg.reshape(NB, P).T.astype(BF16)  # [P, NB]

        # per-slab packing: 32 data tiles then 32 [128,2] masks
        h = np.empty((P, n_slabs, SLAB_COLS), dtype=BF16)
        dslab = data.reshape(n_slabs, SLAB, P, D)
        mslab = masks.reshape(n_slabs, SLAB, P, 2)
        h[:, :, : SLAB * D] = dslab.transpose(2, 0, 1, 3).reshape(
            P, n_slabs, SLAB * D
        )
        h[:, :, SLAB * D :] = mslab.transpose(2, 0, 1, 3).reshape(
            P, n_slabs, SLAB * 2
        )

        iota = np.tile(np.arange(P, dtype=np.float32), (P, 1)).astype(BF16)
        ident = np.eye(P, dtype=np.float32).astype(BF16)
        idx = np.concatenate([iota, ident, runsegT], axis=1)

        recip = (
            1.0
            / np.maximum(
                counts[c * seg_per_core : (c + 1) * seg_per_core], 1.0
            ).astype(np.float32)
        ).astype(np.float32).reshape(P, 1)

        in_maps.append(
            {
                "h": np.ascontiguousarray(h.reshape(P, n_slabs * SLAB_COLS)),
                "idx": np.ascontiguousarray(idx),
                "recip": recip,
            }
        )

    if T not in _prog_cache:
        _prog_cache[T] = _build_program(T)
    nc = _prog_cache[T]

    from concourse.bass_utils import run_bass_kernel_spmd

    trace = bool(os.environ.get("KERNEL_TRACE"))
    result = run_bass_kernel_spmd(
        nc,
        in_maps,
        core_ids=list(range(N_CORES)),
        trace=trace,
        trace_cores=list(range(N_CORES)) if trace else None,
    )
    LAST_RESULT = result

    out = np.concatenate([result.results[c]["out"] for c in range(N_CORES)], axis=0)
    return out.astype(np.float32)


# revision 21
# speedup vs baseline: 1.0132x; 1.0132x over previous
"""Segment-mean (graph pooling) kernel for Trainium2, 8 NeuronCores.

reference semantics:
    sums   = segment_sum(node_h, node_batch, num_segments=G)
    counts = segment_sum(ones(N), node_batch, G)
    out    = sums / max(counts, 1)[:, None]

node_batch is sorted, so segments are contiguous row runs. Sharding:
core c owns segments [128c, 128(c+1)) and streams the node rows that
cover them (a uniform T tiles per core; rows outside the core's
segment range contribute to nothing).

Because rows are sorted, a 128-row tile intersects at most 2 segments
(segments here have ~1953 rows each; host falls back if not). The
reduction runs in two phases so no full [rows, segs] one-hot is ever
built (in the v2 version its DVE is_equal build was the bottleneck at
~265us busy vs the ~180us DMA floor):

  Phase 1 (per tile): matmul with lhsT = the [128 rows, 128 d] bf16
  data tile (stationary) and rhs = [128 rows, 2] 0/1 masks (first-run
  rows / second-run rows, packed into the streamed slab right after
  the data tiles). Out = the tile's two run-sums as two f32 columns
  of a per-64-tile [128 d, 128 runs] PSUM block (tile j -> cols 2j).
  Matmul PSUM writes must start at partition base 0/32/64, so the
  block is built transposed ([d, runs]) and flipped back in phase 2.

  Phase 2 (per 64-tile block, staggered one step per block boundary
  so no engine waits on another): cast block PSUM->SBUF bf16, PE
  transpose via identity -> [runs, d] PSUM, cast -> SBUF bf16, build
  a [128 runs, 128 segs] one-hot from a [128,1] runseg column via one
  DVE is_equal, and accumulate oh.T @ runsumsT into the segment
  accumulator with one matmul. 31 blocks -> ~20us total DVE work.

node_h is cast to bf16 on the host (the ~2^-8 per-element quantization
noise averages down to ~2e-3 relative error on the segment means, well
inside the 2e-2 gate) so HBM traffic is half of f32. Epilogue scales
by 1/max(count,1) computed on host.

The per-instruction ISA limit of ONE semaphore wait shapes the
synchronization: slab DMAs run on the sync (HWDGE) queue, and tiny
gpsimd memset carriers chained on the gpsimd FIFO absorb the
cross-engine WAR waits for slab-buffer and PSUM-block reuse so every
hot-loop instruction needs at most one semaphore wait.
"""

import os

import numpy as np
import ml_dtypes

BF16 = ml_dtypes.bfloat16
P = 128  # partitions / rows per tile / segments per core
D = 128  # feature dim
G = 1024  # num segments
N_CORES = 8
SLAB = 64  # node-tiles per DMA slab
BLK = 64  # tiles per run-sum PSUM block (2 runs/tile -> 128 run cols)
SLABS_PER_BLK = BLK // SLAB
SLAB_COLS = SLAB * D + 2 * SLAB  # data tiles + [128,2] mask per tile
SENTINEL = 200.0  # run seg id outside [0, 128) -> dropped by one-hot
PSUM_BLK_BUFS = 4

_prog_cache: dict[int, object] = {}
LAST_RESULT = None  # BassKernelResults of the most recent device run


def _make_slab_list(T):
    """Uniform SLAB-tile slabs with a shrinking tail (32,16,16) so the
    last slab's un-overlapped compute after the stream ends is short."""
    tail = (SLAB // 2, SLAB // 4, SLAB // 4)
    body = T - sum(tail)
    assert body > 0 and body % SLAB == 0
    slab_list = [(t0, SLAB) for t0 in range(0, body, SLAB)]
    t0 = body
    for n in tail:
        slab_list.append((t0, n))
        t0 += n
    return slab_list


def _np_fallback(node_h, node_batch, num_graphs):
    node_h = np.asarray(node_h, dtype=np.float32)
    nb = np.asarray(node_batch).astype(np.int64)
    ng = int(num_graphs)
    sums = np.zeros((ng, node_h.shape[1]), dtype=np.float32)
    np.add.at(sums, nb, node_h)
    counts = np.bincount(nb, minlength=ng).astype(np.float32)
    return sums / np.maximum(counts, 1.0)[:, None]


def _build_program(T: int):
    import concourse.bacc as bacc
    import concourse.mybir as mybir
    import concourse.tile as tile
    from concourse.tile import add_dep_helper

    bf16 = mybir.dt.bfloat16
    f32 = mybir.dt.float32

    assert T % BLK == 0
    slab_list = _make_slab_list(T)
    NB = T // BLK

    nc = bacc.Bacc(None)
    h_in = nc.dram_tensor("h", [P, T * (D + 2)], bf16, kind="ExternalInput")
    idx_in = nc.dram_tensor("idx", [P, 2 * P + NB], bf16, kind="ExternalInput")
    recip_in = nc.dram_tensor("recip", [P, 1], f32, kind="ExternalInput")
    out_t = nc.dram_tensor("out", [P, D], f32, kind="ExternalOutput")

    with tile.TileContext(nc) as tc:
        with (
            tc.tile_pool(name="const", bufs=1) as constp,
            tc.tile_pool(name="scr", bufs=max(1, len(slab_list))) as scrp,
            tc.tile_pool(name="scr2", bufs=max(1, NB)) as scr2p,
            tc.tile_pool(name="slabs", bufs=7) as slabp,
            tc.tile_pool(name="rs", bufs=3) as rsp,
            tc.tile_pool(name="rst", bufs=3) as rstp,
            tc.tile_pool(name="oh2", bufs=4) as oh2p,
            tc.tile_pool(name="pblk", bufs=PSUM_BLK_BUFS, space="PSUM") as pblkp,
            tc.tile_pool(name="ptr", bufs=2, space="PSUM") as ptrp,
            tc.tile_pool(name="pacc", bufs=1, space="PSUM") as paccp,
            tc.tile_pool(name="outp", bufs=1) as outp,
        ):
            idx_sb = constp.tile([P, 2 * P + NB], bf16)
            nc.gpsimd.dma_start(idx_sb[:], idx_in[:])
            recip_sb = constp.tile([P, 1], f32)
            nc.gpsimd.dma_start(recip_sb[:], recip_in[:])
            iota_sb = idx_sb[:, 0:P]
            ident_sb = idx_sb[:, P : 2 * P]

            acc = paccp.tile([P, D], f32)

            last_mm = {}  # slab idx -> last phase-1 matmul (slab WAR)
            cast1_ins = {}  # block idx -> PSUM->SBUF copy (psum blk WAR)
            rs_tiles = {}
            rst_tiles = {}
            oh2_tiles = {}
            blk_tile = None

            def block_boundary(k):
                """Emitted right after the last phase-1 mm of block k."""
                b_tr = k - 1
                b_mm2 = k - 2
                rs_k = rsp.tile([P, 2 * BLK], bf16)
                cast1_ins[k] = nc.vector.tensor_copy(out=rs_k[:], in_=blk_tile[:])
                rs_tiles[k] = rs_k
                oh2 = oh2p.tile([P, P], bf16)
                nc.vector.tensor_tensor(
                    out=oh2[:],
                    in0=iota_sb,
                    in1=idx_sb[:, 2 * P + k : 2 * P + k + 1].to_broadcast([P, P]),
                    op=mybir.AluOpType.is_equal,
                )
                oh2_tiles[k] = oh2
                if b_tr >= 0:
                    rsT_ps = ptrp.tile([P, 2 * BLK], bf16)
                    nc.tensor.transpose(rsT_ps[:], rs_tiles.pop(b_tr)[:], ident_sb)
                    rsT_sb = rstp.tile([P, 2 * BLK], bf16)
                    nc.vector.tensor_copy(out=rsT_sb[:], in_=rsT_ps[:])
                    rst_tiles[b_tr] = rsT_sb
                if b_mm2 >= 0:
                    nc.tensor.matmul(
                        out=acc[:],
                        lhsT=oh2_tiles.pop(b_mm2)[:],
                        rhs=rst_tiles.pop(b_mm2)[:],
                        start=(b_mm2 == 0),
                        stop=(b_mm2 == NB - 1),
                    )

            for g, (ts0, nt) in enumerate(slab_list):
                slab = slabp.tile([P, SLAB * (D + 2)], bf16)
                car = None
                if g >= 7:
                    scr = scrp.tile([1, 2], f32, name="scr")
                    car = nc.gpsimd.memset(scr[:], 0.0)
                    add_dep_helper(
                        car.ins, last_mm[g - 7].ins, True, "slab WAR carrier"
                    )
                # PSUM-block WAR (first mm of a block vs cast1(b-4)) is left
                # to Tile's reuse tracking: PE has already executed mm2(b-3),
                # which waited on a later DVE count than cast1(b-4), so the
                # hazard is transitively satisfied without gating the DMA.
                c0 = ts0 * (D + 2)
                dma = nc.sync.dma_start(
                    slab[:, : nt * (D + 2)], h_in[:, c0 : c0 + nt * (D + 2)]
                )
                if car is not None:
                    add_dep_helper(dma.ins, car.ins, True, "dma after carrier")

                for i in range(nt):
                    t = ts0 + i
                    b = t // BLK
                    j = t % BLK
                    if j == 0:
                        blk_tile = pblkp.tile([P, 2 * BLK], f32)
                    mm = nc.tensor.matmul(
                        out=blk_tile[:, 2 * j : 2 * j + 2],
                        lhsT=slab[:, i * D : (i + 1) * D],
                        rhs=slab[:, nt * D + 2 * i : nt * D + 2 * i + 2],
                        start=True,
                        stop=True,
                    )
                    if i == nt - 1:
                        last_mm[g] = mm
                    if j == BLK - 1:
                        block_boundary(b)

            # flush the staggered pipeline: transpose/cast for NB-1, then
            # the last two accumulate matmuls
            b_tr = NB - 1
            rsT_ps = ptrp.tile([P, 2 * BLK], bf16)
            nc.tensor.transpose(rsT_ps[:], rs_tiles.pop(b_tr)[:], ident_sb)
            rsT_sb = rstp.tile([P, 2 * BLK], bf16)
            nc.vector.tensor_copy(out=rsT_sb[:], in_=rsT_ps[:])
            rst_tiles[b_tr] = rsT_sb
            for b_mm2 in (NB - 2, NB - 1):
                nc.tensor.matmul(
                    out=acc[:],
                    lhsT=oh2_tiles.pop(b_mm2)[:],
                    rhs=rst_tiles.pop(b_mm2)[:],
                    start=(b_mm2 == 0),
                    stop=(b_mm2 == NB - 1),
                )

            res = outp.tile([P, D], f32)
            nc.vector.tensor_tensor(
                out=res[:],
                in0=acc[:],
                in1=recip_sb[:, 0:1].to_broadcast([P, D]),
                op=mybir.AluOpType.mult,
            )
            nc.sync.dma_start(out_t[:], res[:])

    nc.finalize()
    return nc


def kernel(node_h, node_batch, num_graphs):
    global LAST_RESULT
    node_h = np.asarray(node_h)
    nb = np.asarray(node_batch)
    ng = int(num_graphs)

    N = node_h.shape[0]
    if (
        ng != G
        or node_h.ndim != 2
        or node_h.shape[1] != D
        or nb.shape != (N,)
        or N % P != 0
        or N // P < 4 * SLAB
        or np.any(nb[:-1] > nb[1:])
        or nb[0] < 0
        or nb[-1] >= G
    ):
        return _np_fallback(node_h, node_batch, num_graphs)

    node_h = np.ascontiguousarray(node_h, dtype=np.float32)
    nb = nb.astype(np.int64)

    n_tiles = N // P
    seg_per_core = G // N_CORES
    counts = np.bincount(nb, minlength=G)
    bounds = np.concatenate([[0], np.cumsum(counts)])
    starts = bounds[np.arange(N_CORES) * seg_per_core]
    ends = bounds[(np.arange(N_CORES) + 1) * seg_per_core]
    lo_t = starts // P
    hi_t = -(-ends // P)
    span = int((hi_t - lo_t).max())
    T = ((span + BLK - 1) // BLK) * BLK
    if T > n_tiles:
        return _np_fallback(node_h, node_batch, num_graphs)
    lo = np.minimum(lo_t, n_tiles - T).astype(np.int64)

    # any 128-row tile anywhere spanning >2 distinct segments breaks the
    # 2-runs-per-tile scheme -> fall back (cannot happen for ~1953-row
    # segments)
    seg_all = nb.reshape(n_tiles, P)
    d = seg_all[:, -1] - seg_all[:, 0]
    if np.any(d > 1):
        for t in np.where(d > 1)[0]:
            if len(np.unique(seg_all[t])) > 2:
                return _np_fallback(node_h, node_batch, num_graphs)

    slab_list = _make_slab_list(T)
    NB = T // BLK
    rowidx = np.arange(P)

    in_maps = []
    for c in range(N_CORES):
        r0 = int(lo[c]) * P
        r1 = r0 + T * P
        data = node_h[r0:r1].astype(BF16).reshape(T, P, D)

        seg = (nb[r0:r1] - c * seg_per_core).reshape(T, P)
        first = seg[:, 0]
        last = seg[:, -1]
        k = (seg == first[:, None]).sum(axis=1)  # rows in first run
        mask0 = (rowidx[None, :] < k[:, None]).astype(BF16)  # [T, P]
        masks = np.empty((T, P, 2), dtype=BF16)
        masks[:, :, 0] = mask0
        masks[:, :, 1] = (1.0 - mask0.astype(np.float32)).astype(BF16)

        in_range_f = (first >= 0) & (first < P)
        in_range_l = (last >= 0) & (last < P)
        runseg = np.empty(2 * T, dtype=np.float32)
        runseg[0::2] = np.where(in_range_f, first, SENTINEL)
        runseg[1::2] = np.where((last != first) & in_range_l, last, SENTINEL)
        runsegT = runseg.reshape(NB, P).T.astype(BF16)  # [P, NB]

        # per-slab packing: nt data tiles then nt [128,2] masks
        h = np.empty((P, T * (D + 2)), dtype=BF16)
        for ts0, nt in slab_list:
            c0 = ts0 * (D + 2)
            h[:, c0 : c0 + nt * D] = (
                data[ts0 : ts0 + nt].transpose(1, 0, 2).reshape(P, nt * D)
            )
            h[:, c0 + nt * D : c0 + nt * (D + 2)] = (
                masks[ts0 : ts0 + nt].transpose(1, 0, 2).reshape(P, nt * 2)
            )

        iota = np.tile(np.arange(P, dtype=np.float32), (P, 1)).astype(BF16)
        ident = np.eye(P, dtype=np.float32).astype(BF16)
        idx = np.concatenate([iota, ident, runsegT], axis=1)

        recip = (
            1.0
            / np.maximum(
                counts[c * seg_per_core : (c + 1) * seg_per_core], 1.0
            ).astype(np.float32)
        ).astype(np.float32).reshape(P, 1)

        in_maps.append(
            {
                "h": np.ascontiguousarray(h),
                "idx": np.ascontiguousarray(idx),
                "recip": recip,
            }
        )

    if T not in _prog_cache:
        _prog_cache[T] = _build_program(T)
    nc = _prog_cache[T]

    from concourse.bass_utils import run_bass_kernel_spmd

    trace = bool(os.environ.get("KERNEL_TRACE"))
    result = run_bass_kernel_spmd(
        nc,
        in_maps,
        core_ids=list(range(N_CORES)),
        trace=trace,
        trace_cores=list(range(N_CORES)) if trace else None,
    )
    LAST_RESULT = result

    out = np.concatenate([result.results[c]["out"] for c in range(N_CORES)], axis=0)
    return out.astype(np.float32)


# revision 22
# speedup vs baseline: 1.0181x; 1.0049x over previous
"""Segment-mean (graph pooling) kernel for Trainium2, 8 NeuronCores.

reference semantics:
    sums   = segment_sum(node_h, node_batch, num_segments=G)
    counts = segment_sum(ones(N), node_batch, G)
    out    = sums / max(counts, 1)[:, None]

node_batch is sorted, so segments are contiguous row runs. Sharding:
core c owns segments [128c, 128(c+1)) and streams the node rows that
cover them (a uniform T tiles per core; rows outside the core's
segment range contribute to nothing).

Because rows are sorted, a 128-row tile intersects at most 2 segments
(segments here have ~1953 rows each; host falls back if not). The
reduction runs in two phases so no full [rows, segs] one-hot is ever
built (in the v2 version its DVE is_equal build was the bottleneck at
~265us busy vs the ~180us DMA floor):

  Phase 1 (per tile): matmul with lhsT = the [128 rows, 128 d] bf16
  data tile (stationary) and rhs = [128 rows, 2] 0/1 masks (first-run
  rows / second-run rows, packed into the streamed slab right after
  the data tiles). Out = the tile's two run-sums as two f32 columns
  of a per-64-tile [128 d, 128 runs] PSUM block (tile j -> cols 2j).
  Matmul PSUM writes must start at partition base 0/32/64, so the
  block is built transposed ([d, runs]) and flipped back in phase 2.

  Phase 2 (per 64-tile block, staggered one step per block boundary
  so no engine waits on another): cast block PSUM->SBUF bf16, PE
  transpose via identity -> [runs, d] PSUM, cast -> SBUF bf16, build
  a [128 runs, 128 segs] one-hot from a [128,1] runseg column via one
  DVE is_equal, and accumulate oh.T @ runsumsT into the segment
  accumulator with one matmul. 31 blocks -> ~20us total DVE work.

node_h is cast to bf16 on the host (the ~2^-8 per-element quantization
noise averages down to ~2e-3 relative error on the segment means, well
inside the 2e-2 gate) so HBM traffic is half of f32. Epilogue scales
by 1/max(count,1) computed on host.

The per-instruction ISA limit of ONE semaphore wait shapes the
synchronization: slab DMAs run on the sync (HWDGE) queue, and tiny
gpsimd memset carriers chained on the gpsimd FIFO absorb the
cross-engine WAR waits for slab-buffer and PSUM-block reuse so every
hot-loop instruction needs at most one semaphore wait.
"""

import os

import numpy as np
import ml_dtypes

BF16 = ml_dtypes.bfloat16
P = 128  # partitions / rows per tile / segments per core
D = 128  # feature dim
G = 1024  # num segments
N_CORES = 8
SLAB = 64  # node-tiles per DMA slab
BLK = 64  # tiles per run-sum PSUM block (2 runs/tile -> 128 run cols)
SLABS_PER_BLK = BLK // SLAB
SLAB_COLS = SLAB * D + 2 * SLAB  # data tiles + [128,2] mask per tile
SENTINEL = 200.0  # run seg id outside [0, 128) -> dropped by one-hot
PSUM_BLK_BUFS = 4

_prog_cache: dict[int, object] = {}
LAST_RESULT = None  # BassKernelResults of the most recent device run


def _make_slab_list(T):
    """SLAB-tile slabs with a shrinking tail so the last slab's
    un-overlapped compute after the stream ends is short."""
    slab_list = []
    t0 = 0
    rem = T
    while rem > SLAB:
        slab_list.append((t0, SLAB))
        t0 += SLAB
        rem -= SLAB
    if rem > 32:
        tail = [rem - 32, 16, 16]
    elif rem > 16:
        tail = [rem - 16, 16]
    else:
        tail = [rem]
    for n in tail:
        slab_list.append((t0, n))
        t0 += n
    assert t0 == T and all(0 < n <= SLAB for _, n in slab_list)
    return slab_list


def _np_fallback(node_h, node_batch, num_graphs):
    node_h = np.asarray(node_h, dtype=np.float32)
    nb = np.asarray(node_batch).astype(np.int64)
    ng = int(num_graphs)
    sums = np.zeros((ng, node_h.shape[1]), dtype=np.float32)
    np.add.at(sums, nb, node_h)
    counts = np.bincount(nb, minlength=ng).astype(np.float32)
    return sums / np.maximum(counts, 1.0)[:, None]


def _build_program(T: int):
    import concourse.bacc as bacc
    import concourse.mybir as mybir
    import concourse.tile as tile
    from concourse.tile import add_dep_helper

    bf16 = mybir.dt.bfloat16
    f32 = mybir.dt.float32

    slab_list = _make_slab_list(T)
    NB = -(-T // BLK)  # last block may be partial

    nc = bacc.Bacc(None)
    h_in = nc.dram_tensor("h", [P, T * (D + 2)], bf16, kind="ExternalInput")
    idx_in = nc.dram_tensor("idx", [P, 2 * P + NB], bf16, kind="ExternalInput")
    recip_in = nc.dram_tensor("recip", [P, 1], f32, kind="ExternalInput")
    out_t = nc.dram_tensor("out", [P, D], f32, kind="ExternalOutput")

    with tile.TileContext(nc) as tc:
        with (
            tc.tile_pool(name="const", bufs=1) as constp,
            tc.tile_pool(name="scr", bufs=max(1, len(slab_list))) as scrp,
            tc.tile_pool(name="scr2", bufs=max(1, NB)) as scr2p,
            tc.tile_pool(name="slabs", bufs=7) as slabp,
            tc.tile_pool(name="rs", bufs=3) as rsp,
            tc.tile_pool(name="rst", bufs=3) as rstp,
            tc.tile_pool(name="oh2", bufs=4) as oh2p,
            tc.tile_pool(name="pblk", bufs=PSUM_BLK_BUFS, space="PSUM") as pblkp,
            tc.tile_pool(name="ptr", bufs=2, space="PSUM") as ptrp,
            tc.tile_pool(name="pacc", bufs=1, space="PSUM") as paccp,
            tc.tile_pool(name="outp", bufs=1) as outp,
        ):
            idx_sb = constp.tile([P, 2 * P + NB], bf16)
            nc.gpsimd.dma_start(idx_sb[:], idx_in[:])
            recip_sb = constp.tile([P, 1], f32)
            nc.gpsimd.dma_start(recip_sb[:], recip_in[:])
            iota_sb = idx_sb[:, 0:P]
            ident_sb = idx_sb[:, P : 2 * P]

            acc = paccp.tile([P, D], f32)

            last_mm = {}  # slab idx -> last phase-1 matmul (slab WAR)
            cast1_ins = {}  # block idx -> PSUM->SBUF copy (psum blk WAR)
            rs_tiles = {}
            rst_tiles = {}
            oh2_tiles = {}
            blk_tile = None

            def runs_of(k):
                # run count of block k (last block may be partial)
                return 2 * min(BLK, T - k * BLK)

            def do_transpose(b_tr):
                R = runs_of(b_tr)
                rsT_ps = ptrp.tile([P, 2 * BLK], bf16)
                nc.tensor.transpose(
                    rsT_ps[:, 0:P][0:R, :], rs_tiles.pop(b_tr)[:], ident_sb
                )
                rsT_sb = rstp.tile([P, 2 * BLK], bf16)
                nc.vector.tensor_copy(
                    out=rsT_sb[:, 0:P][0:R, :], in_=rsT_ps[:, 0:P][0:R, :]
                )
                rst_tiles[b_tr] = rsT_sb

            def do_mm2(b_mm2):
                R = runs_of(b_mm2)
                nc.tensor.matmul(
                    out=acc[:],
                    lhsT=oh2_tiles.pop(b_mm2)[:][0:R, :],
                    rhs=rst_tiles.pop(b_mm2)[:, 0:P][0:R, :],
                    start=(b_mm2 == 0),
                    stop=(b_mm2 == NB - 1),
                )

            def block_boundary(k):
                """Emitted right after the last phase-1 mm of block k."""
                R = runs_of(k)
                rs_k = rsp.tile([P, 2 * BLK], bf16)
                cast1_ins[k] = nc.vector.tensor_copy(
                    out=rs_k[:, 0:R], in_=blk_tile[:, 0:R]
                )
                rs_tiles[k] = rs_k[:, 0:R]
                oh2 = oh2p.tile([P, P], bf16)
                nc.vector.tensor_tensor(
                    out=oh2[0:R, :],
                    in0=iota_sb[0:R, :],
                    in1=idx_sb[0:R, 2 * P + k : 2 * P + k + 1].to_broadcast(
                        [R, P]
                    ),
                    op=mybir.AluOpType.is_equal,
                )
                oh2_tiles[k] = oh2
                if k - 1 >= 0:
                    do_transpose(k - 1)
                if k - 2 >= 0:
                    do_mm2(k - 2)

            for g, (ts0, nt) in enumerate(slab_list):
                slab = slabp.tile([P, SLAB * (D + 2)], bf16)
                car = None
                if g >= 7:
                    scr = scrp.tile([1, 2], f32, name="scr")
                    car = nc.gpsimd.memset(scr[:], 0.0)
                    add_dep_helper(
                        car.ins, last_mm[g - 7].ins, True, "slab WAR carrier"
                    )
                # PSUM-block WAR (first mm of a block vs cast1(b-4)) is left
                # to Tile's reuse tracking: PE has already executed mm2(b-3),
                # which waited on a later DVE count than cast1(b-4), so the
                # hazard is transitively satisfied without gating the DMA.
                c0 = ts0 * (D + 2)
                dma = nc.sync.dma_start(
                    slab[:, : nt * (D + 2)], h_in[:, c0 : c0 + nt * (D + 2)]
                )
                if car is not None:
                    add_dep_helper(dma.ins, car.ins, True, "dma after carrier")

                for i in range(nt):
                    t = ts0 + i
                    b = t // BLK
                    j = t % BLK
                    if j == 0:
                        blk_tile = pblkp.tile([P, 2 * BLK], f32)
                    mm = nc.tensor.matmul(
                        out=blk_tile[:, 2 * j : 2 * j + 2],
                        lhsT=slab[:, i * D : (i + 1) * D],
                        rhs=slab[:, nt * D + 2 * i : nt * D + 2 * i + 2],
                        start=True,
                        stop=True,
                    )
                    if i == nt - 1:
                        last_mm[g] = mm
                    if j == BLK - 1 or t == T - 1:
                        block_boundary(b)

            # flush the staggered pipeline: transpose/cast for NB-1, then
            # the last two accumulate matmuls
            do_transpose(NB - 1)
            do_mm2(NB - 2)
            do_mm2(NB - 1)

            res = outp.tile([P, D], f32)
            nc.vector.tensor_tensor(
                out=res[:],
                in0=acc[:],
                in1=recip_sb[:, 0:1].to_broadcast([P, D]),
                op=mybir.AluOpType.mult,
            )
            nc.sync.dma_start(out_t[:], res[:])

    nc.finalize()
    return nc


def kernel(node_h, node_batch, num_graphs):
    global LAST_RESULT
    node_h = np.asarray(node_h)
    nb = np.asarray(node_batch)
    ng = int(num_graphs)

    N = node_h.shape[0]
    if (
        ng != G
        or node_h.ndim != 2
        or node_h.shape[1] != D
        or nb.shape != (N,)
        or N % P != 0
        or N // P < 4 * SLAB
        or np.any(nb[:-1] > nb[1:])
        or nb[0] < 0
        or nb[-1] >= G
    ):
        return _np_fallback(node_h, node_batch, num_graphs)

    node_h = np.ascontiguousarray(node_h, dtype=np.float32)
    nb = nb.astype(np.int64)

    n_tiles = N // P
    seg_per_core = G // N_CORES
    counts = np.bincount(nb, minlength=G)
    bounds = np.concatenate([[0], np.cumsum(counts)])
    starts = bounds[np.arange(N_CORES) * seg_per_core]
    ends = bounds[(np.arange(N_CORES) + 1) * seg_per_core]
    lo_t = starts // P
    hi_t = -(-ends // P)
    T = int((hi_t - lo_t).max())
    if T > n_tiles:
        return _np_fallback(node_h, node_batch, num_graphs)
    lo = np.minimum(lo_t, n_tiles - T).astype(np.int64)

    # any 128-row tile anywhere spanning >2 distinct segments breaks the
    # 2-runs-per-tile scheme -> fall back (cannot happen for ~1953-row
    # segments)
    seg_all = nb.reshape(n_tiles, P)
    d = seg_all[:, -1] - seg_all[:, 0]
    if np.any(d > 1):
        for t in np.where(d > 1)[0]:
            if len(np.unique(seg_all[t])) > 2:
                return _np_fallback(node_h, node_batch, num_graphs)

    slab_list = _make_slab_list(T)
    NB = -(-T // BLK)
    rowidx = np.arange(P)

    in_maps = []
    for c in range(N_CORES):
        r0 = int(lo[c]) * P
        r1 = r0 + T * P
        data = node_h[r0:r1].astype(BF16).reshape(T, P, D)

        seg = (nb[r0:r1] - c * seg_per_core).reshape(T, P)
        first = seg[:, 0]
        last = seg[:, -1]
        k = (seg == first[:, None]).sum(axis=1)  # rows in first run
        mask0 = (rowidx[None, :] < k[:, None]).astype(BF16)  # [T, P]
        masks = np.empty((T, P, 2), dtype=BF16)
        masks[:, :, 0] = mask0
        masks[:, :, 1] = (1.0 - mask0.astype(np.float32)).astype(BF16)

        in_range_f = (first >= 0) & (first < P)
        in_range_l = (last >= 0) & (last < P)
        runseg = np.full(NB * P, SENTINEL, dtype=np.float32)
        runseg[0 : 2 * T : 2] = np.where(in_range_f, first, SENTINEL)
        runseg[1 : 2 * T : 2] = np.where(
            (last != first) & in_range_l, last, SENTINEL
        )
        runsegT = runseg.reshape(NB, P).T.astype(BF16)  # [P, NB]

        # per-slab packing: nt data tiles then nt [128,2] masks
        h = np.empty((P, T * (D + 2)), dtype=BF16)
        for ts0, nt in slab_list:
            c0 = ts0 * (D + 2)
            h[:, c0 : c0 + nt * D] = (
                data[ts0 : ts0 + nt].transpose(1, 0, 2).reshape(P, nt * D)
            )
            h[:, c0 + nt * D : c0 + nt * (D + 2)] = (
                masks[ts0 : ts0 + nt].transpose(1, 0, 2).reshape(P, nt * 2)
            )

        iota = np.tile(np.arange(P, dtype=np.float32), (P, 1)).astype(BF16)
        ident = np.eye(P, dtype=np.float32).astype(BF16)
        idx = np.concatenate([iota, ident, runsegT], axis=1)

        recip = (
            1.0
            / np.maximum(
                counts[c * seg_per_core : (c + 1) * seg_per_core], 1.0
            ).astype(np.float32)
        ).astype(np.float32).reshape(P, 1)

        in_maps.append(
            {
                "h": np.ascontiguousarray(h),
                "idx": np.ascontiguousarray(idx),
                "recip": recip,
            }
        )

    if T not in _prog_cache:
        _prog_cache[T] = _build_program(T)
    nc = _prog_cache[T]

    from concourse.bass_utils import run_bass_kernel_spmd

    trace = bool(os.environ.get("KERNEL_TRACE"))
    result = run_bass_kernel_spmd(
        nc,
        in_maps,
        core_ids=list(range(N_CORES)),
        trace=trace,
        trace_cores=list(range(N_CORES)) if trace else None,
    )
    LAST_RESULT = result

    out = np.concatenate([result.results[c]["out"] for c in range(N_CORES)], axis=0)
    return out.astype(np.float32)
